# revision 18
# baseline (speedup 1.0000x reference)
"""Multi-head attention (B=4, S=2048, D=512, H=8) on 8 trn2 NeuronCores.

Sharding: core c handles batch b=c//2, head-group g=c%2 (4 heads, 256 of the
512 projection dims). Each core runs the full fused pipeline for its four
heads — QKV projection, scores^T = K_h Q_h^T, exp (softmax numerator),
attn @ V with a folded ones-column producing the softmax denominators,
normalization, and its partial output projection y^T = Wo_slice^T.T @ O^T.
The host sums the two partial y^T per batch and adds the output bias.

The kernel is paced by the ScalarE (ACT) engine, which carries the softmax
exp stream — 16.8M elements/core ≈ 143us at 1 elem/lane/cycle — and is the
per-core roofline. Tile's scheduler is a priority heap keyed on emission
order, so the program is EMITTED in execution order: a step loop where each
step issues one key-tile's score matmuls + exp, and everything else
(projection fillers, attnV chain segments trailing their exp stream by half
a group, normalization, output projection, and the BULK input DMAs — which
would otherwise steal the ~360GB/s DMA fabric from the first score tiles)
is interleaved at the step where its inputs are ready. That keeps the ACT
queue dense from ~13us after start to ~15us before the end, the PE warm
(no >3.4us idle windows, which would drop the HAM clock to 1.2GHz and
double matmul cost), and PSUM inside the 8-bank budget:
  sc 2x[128,1024] (4) + ops 2x[65,512] (2) + ps 2x[128,512] (2).

Softmax denominators: each attnV chain's ones-column row lands on PSUM
partition 64 and is copied (partition shifts must be 32-aligned) onto
partition 32*j of a [97, 512] accumulator pre-filled with 1.0; one DVE
reciprocal + a K=97 selector matmul (zeros at the garbage rows) broadcasts
1/denom to the right 64-partition groups. No DMA on the critical tail.

Inputs arrive host-permuted as [128, sb, dt, 512] so 512KB column-chunks
DMA independently. All attention matmuls run bf16 with fp32 PSUM
accumulation; scores^T is computed transposed (keys on partitions) so
exp'd tiles feed the V contraction with no transposes. exp skips
max-subtraction: scaled scores are ~N(0,1), far inside fp32 exp range.
"""

import re

import numpy as np
import ml_dtypes

import concourse.bass as bass
import concourse.mybir as mybir
from concourse.bass_utils import run_bass_kernel_spmd
from concourse.tile import ScopedClock, TileContext, VectorClock

BF16 = mybir.dt.bfloat16
F32 = mybir.dt.float32
F32R = mybir.dt.float32r
NP_BF16 = ml_dtypes.bfloat16

B, S, D, H, DK = 4, 2048, 512, 8, 64
SCALE = float(1.0 / (np.float32(np.sqrt(DK)) + 1e-8))
E = 256          # head dims per core (4 heads)
NCORES = 8
KT = S // 128    # 16 key tiles of 128
SB = S // 512    # 4 s-blocks of 512
NG = 4           # (q-block, head-pair) groups: g -> qb=g>>1, hp=g&1


# ---------------------------------------------------------------------------
# walrus in this container rejects >1 sync-wait command per instruction;
# split the Tile tail drain and hoist excess mid-kernel waits onto NoOps.
# ---------------------------------------------------------------------------

def _clock_entries(vc):
    nums = [int(s) for s in re.findall(r"-?\d+", repr(vc))]
    return [(i, n) for i, n in enumerate(nums) if n > 0]


class SplitDrainTileContext(TileContext):
    def _drain_and_barrier(self, tick_clock, wait_clock):
        nc = self.nc
        for proc, tick in _clock_entries(tick_clock.global_clock):
            vc = VectorClock()
            vc.require_at_least(proc, tick)
            carrier = nc.sync.nop()
            wait_clock.add_sem_waits(carrier.ins, ScopedClock({None: vc}))
        nc.sync.drain()
        nc.all_engine_barrier()
        assert self.sems is not None
        popped = nc._tile_sem_poison_stack.pop()
        assert popped is self._sem_poison
        nc.clear_and_free_semaphores(list(self.sems.allocated().values()))
        nc.all_engine_barrier()


def sanitize_waits(nc, max_waits: int = 1):
    n_split = 0
    for fn in nc.m.functions:
        for bb in fn.blocks:
            new_insts = []
            for inst in bb.instructions:
                si = inst.sync_info
                waits = list(si.on_wait) if si and si.on_wait else []
                if len(waits) > max_waits:
                    keep = waits[-max_waits:]
                    excess = waits[:-max_waits]
                    for i in range(0, len(excess), max_waits):
                        nop = mybir.InstNoOp(
                            name=nc.get_next_instruction_name(), ins=[], outs=[]
                        )
                        nop.engine = inst.engine
                        nop.sync_info = mybir.SyncInfo(
                            on_wait=excess[i : i + max_waits], on_update=[]
                        )
                        new_insts.append(nop)
                    inst.sync_info = mybir.SyncInfo(
                        on_wait=keep, on_update=si.on_update
                    )
                    n_split += 1
                new_insts.append(inst)
            bb.instructions[:] = new_insts
    return n_split


# ---------------------------------------------------------------------------
# kernel builder (one SPMD program; per-core data differs only in in_maps)
# ---------------------------------------------------------------------------

def build_nc(sanitize=True):
    nc = bass.Bass("TRN2", target_bir_lowering=False, debug=False,
                   num_devices=NCORES)

    xqT = nc.declare_dram_parameter("xqT", [128, SB, 4, 512], BF16, isOutput=False)
    xkT = nc.declare_dram_parameter("xkT", [128, SB, 4, 512], BF16, isOutput=False)
    xvT = nc.declare_dram_parameter("xvT", [128, SB, 4, 512], BF16, isOutput=False)
    wqT = nc.declare_dram_parameter("wqT", [128, 4, E], BF16, isOutput=False)
    wkT = nc.declare_dram_parameter("wkT", [128, 4, E], BF16, isOutput=False)
    wvT = nc.declare_dram_parameter("wvT", [128, 4, E], BF16, isOutput=False)
    woT = nc.declare_dram_parameter("woT", [128, 2, D], BF16, isOutput=False)
    bqs = nc.declare_dram_parameter("bqs", [E], F32, isOutput=False)
    bks = nc.declare_dram_parameter("bks", [E], F32, isOutput=False)
    bvb = nc.declare_dram_parameter("bvb", [128, E], F32, isOutput=False)
    # E97[k, sq*128+m] selector: row 32*(hh*2+sq) of the [97, 512]
    # reciprocal tile is broadcast onto partitions hh*64..hh*64+63 of the
    # q-chunk sq via a K=97 matmul; all other (garbage) rows hit zeros.
    e2d = nc.declare_dram_parameter("e2d", [97, 256], F32R, isOutput=False)
    yT = nc.declare_dram_parameter("yT", [D, S], F32, isOutput=True)

    Exp = mybir.ActivationFunctionType.Exp

    with SplitDrainTileContext(nc) as tc:
        with tc.sbuf_pool(name="persist", bufs=1) as P, \
             tc.sbuf_pool(name="ptp", bufs=36) as PTP, \
             tc.sbuf_pool(name="nrm", bufs=1) as NRM, \
             tc.sbuf_pool(name="yo", bufs=4) as YO, \
             tc.psum_pool(name="scp", bufs=2) as SCP, \
             tc.psum_pool(name="opp", bufs=2) as OPP, \
             tc.psum_pool(name="aux", bufs=2) as AUXP:
            QT = P.tile([128, 2, S], BF16)    # e-tiles x queries
            KTt = P.tile([128, 2, S], BF16)
            VA = P.tile([128, KT, 4 * 65], BF16)  # [V_h | ones] per head
            OT = P.tile([128, 2, S], BF16)    # normalized O^T
            WOT = P.tile([128, 2, D], BF16)
            BQ = P.tile([128, 2], F32)
            BK = P.tile([128, 2], F32)
            BVB = P.tile([128, E], F32)
            E97 = P.tile([97, 256], F32R)
            S97 = [P.tile([97, 512], F32, name=f"s97_{i}") for i in range(2)]
            XQT = P.tile([128, SB, 4, 512], BF16)
            XKT = P.tile([128, SB, 4, 512], BF16)
            XVT = P.tile([128, SB, 4, 512], BF16)
            WQ = P.tile([128, 4, E], BF16)
            WK = P.tile([128, 4, E], BF16)
            WVs = P.tile([128, 4, E], BF16)

            # ---- critical-path DMAs only; bulk is deferred into the step
            # loop so these ~1.8MB get the whole ~360GB/s DMA fabric.
            # Per-queue order IS the transfer order (each HWDGE/SWDGE queue
            # is FIFO; issue instructions all fire at t~7us). Critical path
            # first on the two fast HWDGE queues, bulk behind it; the slow
            # SWDGE (gpsimd) queue only carries what isn't needed early.
            nc.sync.dma_start(out=XKT[:, 0], in_=xkT[:, 0])
            nc.sync.dma_start(out=XQT[:, 1], in_=xqT[:, 1])
            for sb in range(1, SB):
                nc.sync.dma_start(out=XKT[:, sb], in_=xkT[:, sb])
            nc.sync.dma_start(out=XQT[:, 2], in_=xqT[:, 2])
            nc.sync.dma_start(out=XQT[:, 3], in_=xqT[:, 3])
            nc.scalar.dma_start(out=WK[:, :, :], in_=wkT[:, :, :])
            nc.scalar.dma_start(out=WQ[:, :, :], in_=wqT[:, :, :])
            nc.scalar.dma_start(out=XQT[:, 0], in_=xqT[:, 0])
            nc.scalar.dma_start(out=XVT[:, 0], in_=xvT[:, 0])
            nc.scalar.dma_start(out=XVT[:, 1], in_=xvT[:, 1])
            nc.gpsimd.dma_start(
                out=BQ[:, :], in_=bqs[:].rearrange("(c p) -> p c", p=128)
            )
            nc.gpsimd.dma_start(
                out=BK[:, :], in_=bks[:].rearrange("(c p) -> p c", p=128)
            )
            nc.gpsimd.dma_start(out=BVB[:, :], in_=bvb[:, :])
            nc.gpsimd.dma_start(out=WVs[:, :, :], in_=wvT[:, :, :])
            nc.gpsimd.dma_start(out=XVT[:, 2], in_=xvT[:, 2])
            nc.gpsimd.dma_start(out=XVT[:, 3], in_=xvT[:, 3])
            nc.gpsimd.dma_start(out=E97[:, :], in_=e2d[:, :])
            nc.gpsimd.dma_start(out=WOT[:, :, :], in_=woT[:, :, :])


            # PE warm-up: HAM starts at K=4/8 (1.2GHz) and needs ~3.4us of
            # sustained matmul activity to unthrottle. Dep-free dummy MMs
            # run during the input-DMA wait so the projections start warm.
            WRM = P.tile([128, 128], BF16)
            nc.vector.memset(WRM[:, :], 0.0)
            wps = AUXP.tile([128, 128], F32, tag="ps", name="wps",
                            padded_shape=[128, 512])
            for _ in range(55):
                nc.tensor.matmul(wps[:, :], lhsT=WRM[:, :], rhs=WRM[:, :],
                                 start=True, stop=True)

            # ---- emission helpers ----------------------------------------
            def proj_qk(which, et, sb):
                xt, wt, out, bias = (
                    (XKT, WK, KTt, BK) if which == "K" else (XQT, WQ, QT, BQ)
                )
                ssl = slice(sb * 512, (sb + 1) * 512)
                ps = AUXP.tile([128, 512], F32, tag="ps")
                for dt in range(4):
                    nc.tensor.matmul(
                        ps[:, :],
                        lhsT=wt[:, dt, et * 128:(et + 1) * 128],
                        rhs=xt[:, sb, dt, :],
                        start=(dt == 0),
                        stop=(dt == 3),
                    )
                nc.vector.tensor_scalar_add(
                    out[:, et, ssl], ps[:, :], bias[:, et:et + 1]
                )

            def proj_v(kt):
                ps = AUXP.tile([128, 512], F32, tag="ps", name="psv")
                psv = ps[:, 0:E]
                sb, ko = kt // 4, (kt % 4) * 128
                for dt in range(4):
                    nc.tensor.matmul(
                        psv,
                        lhsT=XVT[:, sb, dt, ko:ko + 128],
                        rhs=WVs[:, dt, :],
                        start=(dt == 0),
                        stop=(dt == 3),
                    )
                va_h = VA[:, kt, :].rearrange("p (h c) -> p h c", c=65)
                nc.vector.tensor_add(
                    va_h[:, :, 0:64],
                    psv.rearrange("p (h c) -> p h c", c=64),
                    BVB[:, :].rearrange("p (h c) -> p h c", c=64),
                )

            pts = {}       # (g, hh, kt) -> pt tile
            copp = {}      # (g, hh, sq) -> ops accumulator
            ous = {}       # (g, j) -> unnormalized O (+denominator row)
            rcrs = {}      # g -> reciprocal broadcast source [97, 512] f32r

            def sc_step(g, kt):
                qb, hp = g >> 1, g & 1
                q0 = qb * 1024
                scs = [SCP.tile([128, 1024], F32, tag="sc", name=f"sc{hh}")
                       for hh in range(2)]
                for hf in range(2):
                    for hh in range(2):   # row-group pairs run concurrently
                        hsl = slice(hh * 64, hh * 64 + 64)
                        nc.tensor.matmul(
                            scs[hh][:, hf * 512:(hf + 1) * 512],
                            lhsT=KTt[hsl, hp, kt * 128:(kt + 1) * 128],
                            rhs=QT[hsl, hp,
                                   q0 + hf * 512:q0 + (hf + 1) * 512],
                            start=True,
                            stop=True,
                        )
                for hh in range(2):
                    pt = PTP.tile([128, 1024], BF16, tag="pt")
                    if g == 0 and kt == 0:
                        for hf in range(2):
                            hfs = slice(hf * 512, (hf + 1) * 512)
                            nc.scalar.activation(pt[:, hfs], scs[hh][:, hfs],
                                                 Exp, scale=SCALE)
                    else:
                        nc.scalar.activation(pt[:, :], scs[hh][:, :], Exp,
                                             scale=SCALE)
                    pts[g, hh, kt] = pt

            def chain_mms(g, hh, kts, aux=False):
                h = (g & 1) * 2 + hh
                for kt in kts:
                    for sq in range(2):
                        key = (g, hh, sq)
                        if kt == 0:
                            if aux:
                                full = AUXP.tile([128, 512], F32, tag="ps",
                                                 name="opsx")
                                copp[key] = full[0:65, :]
                            else:
                                copp[key] = OPP.tile([65, 512], F32,
                                                     tag="ops", name="ops")
                        nc.tensor.matmul(
                            copp[key][:, :],
                            lhsT=VA[:, kt, h * 65:(h + 1) * 65],
                            rhs=pts[g, hh, kt][:, sq * 512:(sq + 1) * 512],
                            start=(kt == 0),
                            stop=(kt == KT - 1),
                        )

            def chain_drain(g, hh, tail=False):
                for sq in range(2):
                    j = hh * 2 + sq
                    ou = NRM.tile([65, 512], F32, tag="ou", bufs=6)
                    if tail and sq == 1:
                        nc.scalar.copy(ou[:, :], copp[g, hh, sq][:, :])
                    else:
                        nc.vector.tensor_copy(ou[:, :], copp[g, hh, sq][:, :])
                    # denominator row -> partition 32*j of the group's [97,*]
                    # accumulator (32-aligned partition shifts are legal)
                    nc.vector.tensor_copy(
                        S97[g % 2][32 * j:32 * j + 1, :], ou[64:65, :]
                    )
                    ous[g, j] = ou

            def norm_recip(g, tail=False):
                rcr = NRM.tile([97, 512], F32R, tag="rcr", bufs=1, name="rcr")
                if tail:
                    # ACT engine is idle after the last exp; Ln and Exp live
                    # in the same activation-table set, so 1/d = exp(-ln d)
                    # costs ~1.1us there vs 3.35us of DVE iterative divide.
                    rc = NRM.tile([97, 512], F32, tag="rc", bufs=1)
                    nc.scalar.activation(rc[:, :], S97[g % 2][:, :],
                                         mybir.ActivationFunctionType.Ln)
                    with nc.allow_low_precision(
                        reason="softmax 1/denom rounded to fp32r for the "
                        "selector-matmul broadcast"
                    ):
                        nc.scalar.activation(rcr[:, :], rc[:, :], Exp,
                                             scale=-1.0)
                else:
                    rc = NRM.tile([97, 512], F32, tag="rc", bufs=1)
                    nc.vector.reciprocal(rc[:, :], S97[g % 2][:, :])
                    with nc.allow_low_precision(
                        reason="softmax 1/denom rounded to fp32r for the "
                        "selector-matmul broadcast"
                    ):
                        nc.vector.tensor_copy(rcr[:, :], rc[:, :])
                rcrs[g] = rcr

            def norm_bc(g, sq, tail=False):
                qb, hp = g >> 1, g & 1
                s0 = qb * 1024 + sq * 512
                ssl = slice(s0, s0 + 512)
                bc = AUXP.tile([128, 512], F32, tag="ps", name="bc")
                nc.tensor.matmul(
                    bc[:, :],
                    lhsT=E97[:, sq * 128:(sq + 1) * 128],
                    rhs=rcrs[g][:, :],
                    start=True, stop=True,
                )
                nc.vector.tensor_mul(
                    OT[0:64, hp, ssl], ous[g, sq][0:64, :], bc[0:64, :]
                )
                nc.vector.tensor_mul(
                    OT[64:128, hp, ssl], ous[g, 2 + sq][0:64, :],
                    bc[64:128, :],
                )

            def outproj(qb, sq, tail=False):
                s0 = qb * 1024 + sq * 512
                ssl = slice(s0, s0 + 512)
                for fc in range(4):
                    yp = AUXP.tile([128, 512], F32, tag="ps", name="yp")
                    for et in range(2):
                        nc.tensor.matmul(
                            yp[:, :],
                            lhsT=WOT[:, et, fc * 128:(fc + 1) * 128],
                            rhs=OT[:, et, ssl],
                            start=(et == 0),
                            stop=(et == 1),
                        )
                    ys = YO.tile([128, 512], F32, tag="ys")
                    if tail:
                        # spread tail copies over the idle ACT engine + DVE
                        if fc % 2 == 0:
                            nc.scalar.copy(ys[:, :], yp[:, :])
                        else:
                            nc.vector.tensor_copy(ys[:, :], yp[:, :])
                        eng = (nc.sync, nc.scalar, nc.sync, nc.scalar)[fc]
                    else:
                        nc.vector.tensor_copy(ys[:, :], yp[:, :])
                        eng = nc.sync
                    eng.dma_start(
                        out=yT[fc * 128:(fc + 1) * 128, s0:s0 + 512],
                        in_=ys[:, :],
                    )

            # ---- the schedule --------------------------------------------
            proj_qk("K", 0, 0)
            proj_qk("Q", 0, 0)
            proj_qk("Q", 0, 1)
            KQ0 = {2: ("K", 0, 1), 3: ("K", 0, 2), 4: ("K", 0, 3),
                   5: ("K", 1, 0), 6: ("K", 1, 1), 7: ("K", 1, 2),
                   8: ("K", 1, 3), 9: ("Q", 1, 0), 10: ("Q", 1, 1),
                   11: ("Q", 0, 2), 12: ("Q", 0, 3),
                   13: ("Q", 1, 2), 14: ("Q", 1, 3)}
            KQ1 = {}

            for g in range(NG + 1):
                for s in range(KT):
                    if g == 0 and s in (0, 1):
                        # ones columns of V_aug, needed by chains from s8
                        for kt in range(8 * s, 8 * s + 8):
                            va_h = VA[:, kt, :].rearrange(
                                "p (h c) -> p h c", c=65)
                            nc.vector.memset(va_h[:, :, 64:65], 1.0)
                    if g == 0 and s == 5:
                        for i in range(2):
                            # garbage rows stay 1.0 forever: 1/1.0 is finite
                            # and the selector matmul zeroes it; avoids
                            # NaN*0 in the broadcast
                            nc.vector.memset(S97[i][:, :], 1.0)
                    if g < NG:
                        sc_step(g, s)
                    if g == 0:
                        if 3 <= s <= 10:
                            proj_v(2 * (s - 3))
                            proj_v(2 * (s - 3) + 1)
                        if s in KQ0:
                            proj_qk(*KQ0[s])
                    if g == 1 and s in KQ1:
                        proj_qk(*KQ1[s])
                    if g >= 1 and s <= 7 and not (g == NG):
                        # finish group g-1: its hh=1 chains (pts buffered)
                        chain_mms(g - 1, 1, [2 * s, 2 * s + 1])
                        if s == 7:
                            chain_drain(g - 1, 1)
                            norm_recip(g - 1)
                    if g == NG and s == 0:
                        # last group's hh=1 chains: kt 0-7 ran inside g3 in
                        # the freed aux banks; kt 8-15 remain here
                        chain_mms(NG - 1, 1, list(range(8, KT)), aux=True)
                        chain_drain(NG - 1, 1, tail=True)
                        norm_recip(NG - 1, tail=True)
                        # dep-free N=512 matmuls bridge the PE through the
                        # normalization wait so outproj runs at K=8/8
                        wbr = OPP.tile([65, 512], F32, tag="ops", name="wbr")
                        for _ in range(25):
                            nc.tensor.matmul(wbr[0:64, :],
                                             lhsT=WRM[:, 0:64],
                                             rhs=QT[:, 0, 0:512],
                                             start=True, stop=True)
                    if g == NG - 1 and 12 <= s <= 15:
                        chain_mms(NG - 1, 1, [2 * (s - 12), 2 * (s - 12) + 1],
                                  aux=True)
                    if g < NG and s >= 8:
                        # this group's hh=0 chains trail its exp stream
                        chain_mms(g, 0, [2 * (s - 8), 2 * (s - 8) + 1])
                        if s == 15:
                            chain_drain(g, 0)
                    if g >= 1:
                        if s == 10:
                            norm_bc(g - 1, 0, tail=(g == NG))
                        if s == 11:
                            norm_bc(g - 1, 1, tail=(g == NG))
                        if (g - 1) & 1:
                            if s == 12:
                                outproj((g - 1) >> 1, 0, tail=(g == NG))
                            if s == 13:
                                outproj((g - 1) >> 1, 1, tail=(g == NG))

    if sanitize:
        sanitize_waits(nc)
    return nc


def _perm_xt(x):
    # (S, D) -> x^T laid out [128, sb, dt, 512]: partition p, chunk (sb, dt)
    # = row dt*128+p, cols sb*512.. of x^T; 4KB contiguous per partition per
    # sb-chunk.
    xt = np.asarray(x).T.astype(NP_BF16)          # (512, S)
    return np.ascontiguousarray(
        xt.reshape(4, 128, SB, 512).transpose(1, 2, 0, 3)
    )


def _perm_w4(wt):
    # (512, 256) -> [128, 4, 256]: partition p, chunk dt = row dt*128+p
    return np.ascontiguousarray(
        np.asarray(wt).astype(NP_BF16).reshape(4, 128, E).transpose(1, 0, 2)
    )


def _perm_w2(wt):
    # (256, 512) -> [128, 2, 512]: partition p, chunk et = row et*128+p
    return np.ascontiguousarray(
        np.asarray(wt).astype(NP_BF16).reshape(2, 128, D).transpose(1, 0, 2)
    )


def _e97():
    e = np.zeros((97, 256), dtype=np.float32)
    for sq in range(2):
        for hh in range(2):
            e[32 * (hh * 2 + sq), sq * 128 + hh * 64:sq * 128 + hh * 64 + 64] = 1.0
    return e


def make_in_maps(query, key, value, Wq, bq, Wk, bk, Wv, bv, Wo, bo):
    in_maps = []
    for c in range(NCORES):
        b, g = divmod(c, 2)
        eo = g * E
        esl = slice(eo, eo + E)
        in_maps.append({
            "xqT": _perm_xt(query[b]),
            "xkT": _perm_xt(key[b]),
            "xvT": _perm_xt(value[b]),
            "wqT": _perm_w4(Wq[esl, :].T),
            "wkT": _perm_w4(Wk[esl, :].T),
            "wvT": _perm_w4(Wv[esl, :].T),
            "woT": _perm_w2(Wo[:, esl].T),
            "bqs": np.ascontiguousarray(bq[esl], dtype=np.float32),
            "bks": np.ascontiguousarray(bk[esl], dtype=np.float32),
            "bvb": np.ascontiguousarray(
                np.broadcast_to(bv[esl], (128, E)), dtype=np.float32
            ),
            "e2d": _e97(),
        })
    return in_maps


def gather(results, bo):
    out = np.empty((B, S, D), dtype=np.float32)
    for b in range(B):
        yt = results[2 * b]["yT"] + results[2 * b + 1]["yT"]
        out[b] = yt.T + np.asarray(bo, dtype=np.float32)
    return out


_NC = None


def kernel(query, key, value, Wq, bq, Wk, bk, Wv, bv, Wo, bo, **run_kwargs):
    global _NC
    if _NC is None:
        _NC = build_nc()
    args = [np.asarray(a) for a in
            (query, key, value, Wq, bq, Wk, bk, Wv, bv, Wo, bo)]
    in_maps = make_in_maps(*args)
    res = run_bass_kernel_spmd(_NC, in_maps, list(range(NCORES)), **run_kwargs)
    out = gather(res.results, args[10])
    if run_kwargs:
        return out, res
    return out
